# revision 11
# baseline (speedup 1.0000x reference)
"""Trainium2 Bass kernel for nn_CorrelationHead (8-core SPMD, data parallel over B).

Math reformulation (validated ~1e-6 vs the jax reference in fp32):
  corr[b,p,q,i,j] = sum_c patch1[b,c,i,j] * patch2[b,c, i+2p-20, j+2q-20]
  out[b,n] = sum w[n,:]*corr[b,:] + bias[n]
           = sum_{ij,yx} (P1[b]^T P2[b])[ij,yx] * W3[n,ij,yx] + bias[n]
  where W3 gathers w_bbox onto the 49x49 (ij,yx) grid (displacements that
  land outside the 7x7 patch hit zero padding and drop out).

Device mapping per core (64 samples), bf16, raw bass (hand-rolled sems):
  - host packs both patches channel-pair-interleaved: Y[b,p,196] =
    [p1[b,2p] | p2[b,2p] | p1[b,2p+1] | p2[b,2p+1]]  -> 392B-contiguous
    DMA descriptors; loaded as 16 sliced DMAs alternating the two HWDGE
    rings so the PE can chase the stream.
  - stage 1: per sample two accumulating K=64 matmuls (even/odd channel
    halves) -> PSUM A^T[b] [yx=49, ij=49]; 4 samples per PSUM slot-group,
    batch-cast (f32->bf16) to SBUF acat[yx,b,ij], alternating DVE/ACT.
  - stage 2: 49 accumulating matmuls contract ij (lhsT = 4 columns of the
    gathered weights, rhs = acat[:,:,ij]), interleaved over 4 independent
    accumulation chains pinned to distinct PE column strips via
    tile_position=(0,32c) so each strip's weight buffer loads while other
    strips compute; chain partials land at partitions 32c..32c+3.
  - a final selector matmul (0/1 matrix, with the bias folded in via a
    DMA'd ones-row at partition 127) sums the 4 chains across partitions.
  - patches are host-packed 4-samples-per-partition-row (1568B DMA runs)
    and streamed as 16 slices round-robined over three issue queues
    (sync/scalar HWDGE + gpsimd SWDGE); small loads go last on gpsimd.
"""

import numpy as np

import concourse.bass as bass
import concourse.mybir as mybir
from concourse import bacc
from concourse.bass_utils import run_bass_kernel_spmd

N_CORES = 8
B, C, HW = 512, 128, 49
BS = B // N_CORES   # 64 samples per core
CP = C // 2         # 64 partitions, 2 channels each
FW = 4 * HW         # 196 packed row: p1/even, p2/even, p1/odd, p2/odd
PAD = 20
GROUPS = 4
GB = BS // GROUPS   # 16
PSB = 4             # samples per PSUM slot-group
NCHAIN = 4

_F32 = mybir.dt.float32
_BF16 = mybir.dt.bfloat16


def _build_wst2(w_bbox: np.ndarray) -> np.ndarray:
    W3 = np.zeros((4, 49, 49), np.float32)
    for i in range(7):
        for j in range(7):
            for y in range(7):
                for x in range(7):
                    if (y - i) % 2 == 0 and (x - j) % 2 == 0:
                        p = (y - i + PAD) // 2
                        q = (x - j + PAD) // 2
                        W3[:, i * 7 + j, y * 7 + x] = w_bbox[
                            :, ((p * 21 + q) * 7 + i) * 7 + j
                        ]
    return np.ascontiguousarray(W3.transpose(2, 1, 0).reshape(49, 196))


def build_nc() -> bass.Bass:
    nc = bacc.Bacc("TRN2", target_bir_lowering=False, debug=False)
    pp = nc.dram_tensor("pp", [BS // 4, CP, 4 * FW], _BF16, kind="ExternalInput")
    wst2 = nc.dram_tensor("wst2", [49, 196], _BF16, kind="ExternalInput")
    seld = nc.dram_tensor("seld", [128, 4], _BF16, kind="ExternalInput")
    onesd = nc.dram_tensor("onesd", [1, BS], _BF16, kind="ExternalInput")
    out = nc.dram_tensor("out", [4, BS], _F32, kind="ExternalOutput")

    ppr = pp[:].rearrange("b p f -> p b f")

    from contextlib import ExitStack

    with ExitStack() as ctx:
        ts_ = [
            ctx.enter_context(nc.sbuf_tensor(f"t{g}", [CP, GB // 4, 4 * FW], _BF16))
            for g in range(GROUPS)
        ]
        t0, t1, t2, t3 = ts_
        acat = ctx.enter_context(nc.sbuf_tensor("acat", [49, BS, HW], _BF16))
        w_t = ctx.enter_context(nc.sbuf_tensor("w_t", [49, 196], _BF16))
        sel_w = ctx.enter_context(nc.sbuf_tensor("sel_w", [128, 4], _BF16))
        sel_sb = ctx.enter_context(nc.sbuf_tensor("sel_sb", [128, BS], _BF16))
        out_sb = ctx.enter_context(nc.sbuf_tensor("out_sb", [4, BS], _F32))
        ps = ctx.enter_context(nc.psum_tensor("ps", [128, 8, 512], _F32))
        (sW1, sW2, sMM, sCastD, sCastA, sS2, sOut, sDone,
         sW3, sSel, sS3) = (
            ctx.enter_context(nc.semaphore(nm))
            for nm in (
                "sW1", "sW2",
                "sMM", "sCastD", "sCastA", "sS2", "sOut", "sDone",
                "sW3", "sSel", "sS3",
            )
        )
        sD = [
            ctx.enter_context(nc.semaphore(f"sD{i}")) for i in range(16)
        ]
        sS2c = [
            ctx.enter_context(nc.semaphore(f"sS2c{i}")) for i in range(NCHAIN)
        ]
        block = ctx.enter_context(nc.Block())

        @block.sync
        def _(sync):
            for sl in range(0, 16, 3):
                sync.dma_start(
                    out=ts_[sl // 4][:, sl % 4, :],
                    in_=ppr[:, sl, :],
                ).then_inc(sD[sl], 16)
            sync.wait_ge(sOut, 1)
            sync.dma_start(out=out[:], in_=out_sb[:]).then_inc(sDone, 16)
            sync.wait_ge(sDone, 16)

        @block.scalar
        def _(scalar):
            for sl in range(1, 16, 3):
                scalar.dma_start(
                    out=ts_[sl // 4][:, sl % 4, :],
                    in_=ppr[:, sl, :],
                ).then_inc(sD[sl], 16)
            for T in range(1, 16, 2):  # odd slot-groups cast on ACT
                scalar.wait_ge(sMM, T + 1)
                nc.scalar.copy(
                    acat[:, T * PSB : (T + 1) * PSB, :],
                    ps[0:49, (T % 2) * 4 : (T % 2) * 4 + 4, 0:HW],
                ).then_inc(sCastA, 1)

        @block.gpsimd
        def _(gpsimd):
            for sl in range(2, 16, 3):
                gpsimd.dma_start(
                    out=ts_[sl // 4][:, sl % 4, :],
                    in_=ppr[:, sl, :],
                ).then_inc(sD[sl], 16)
            gpsimd.dma_start(out=w_t[:], in_=wst2[:]).then_inc(sW1, 16)
            gpsimd.dma_start(out=sel_w[:], in_=seld[:]).then_inc(sW3, 16)
            gpsimd.dma_start(out=sel_sb[127:128, :], in_=onesd[:]).then_inc(
                sW2, 16
            )

        @block.tensor
        def _(tensor):
            for g in range(GROUPS):
                t = ts_[g]
                for k in range(GB):
                    bb = g * GB + k
                    T, j = bb // PSB, bb % PSB
                    if j == 0:
                        tensor.wait_ge(sD[T], 16)
                    if j == 0 and T >= 2:
                        # reuse of PSUM slot T%2: wait for cast of tile T-2
                        if T % 2 == 0:
                            tensor.wait_ge(sCastD, (T - 2) // 2 + 1)
                        else:
                            tensor.wait_ge(sCastA, (T - 2) // 2 + 1)
                    slot = T % 2
                    q, base = k // 4, (k % 4) * FW
                    nc.tensor.matmul(
                        ps[0:49, slot * 4 + j, 0:HW],
                        t[:, q, base + 49 : base + 98],
                        t[:, q, base : base + 49],
                        start=True,
                        stop=False,
                    )
                    mm2 = nc.tensor.matmul(
                        ps[0:49, slot * 4 + j, 0:HW],
                        t[:, q, base + 147 : base + 196],
                        t[:, q, base + 98 : base + 147],
                        start=False,
                        stop=True,
                    )
                    if j == PSB - 1:
                        mm2.then_inc(sMM, 1)
            # stage 2: needs all of acat + w_t
            tensor.wait_ge(sCastD, 8)
            tensor.wait_ge(sCastA, 8)
            tensor.wait_ge(sW1, 16)
            for ij in range(HW):
                c = ij % NCHAIN
                mm = nc.tensor.matmul(
                    ps[32 * c : 32 * c + 4, c, 0:BS],
                    w_t[:, ij * 4 : (ij + 1) * 4],
                    acat[:, :, ij],
                    start=(ij < NCHAIN),
                    stop=(ij + NCHAIN >= HW),
                    tile_position=(0, 32 * c),
                )
                if ij + NCHAIN >= HW:
                    mm.then_inc(sS2c[c], 1)
            tensor.wait_ge(sSel, 1)
            tensor.wait_ge(sW3, 16)
            tensor.wait_ge(sW2, 16)
            nc.tensor.matmul(
                ps[0:4, 7, 0:BS], sel_w[:], sel_sb[:], start=True, stop=True
            ).then_inc(sS3, 1)

        @block.vector
        def _(vector):
            nc.vector.memset(sel_sb[0:127, :], 0.0)
            for T in range(0, 16, 2):  # even slot-groups cast on DVE
                vector.wait_ge(sMM, T + 1)
                nc.vector.tensor_copy(
                    acat[:, T * PSB : (T + 1) * PSB, :],
                    ps[0:49, (T % 2) * 4 : (T % 2) * 4 + 4, 0:HW],
                ).then_inc(sCastD, 1)
            last_cast = None
            for c in (1, 2, 3, 0):  # chain completion order
                vector.wait_ge(sS2c[c], 1)
                last_cast = nc.vector.tensor_copy(
                    sel_sb[32 * c : 32 * c + 4, :],
                    ps[32 * c : 32 * c + 4, c, 0:BS],
                )
            last_cast.then_inc(sSel, 1)
            vector.wait_ge(sS3, 1)
            nc.vector.tensor_copy(out_sb[:], ps[0:4, 7, 0:BS]).then_inc(sOut, 1)

    nc.compile()
    return nc


def _prep_inputs(inputs):
    import ml_dtypes

    p1 = np.asarray(inputs["patch1"], np.float32).reshape(B, C, HW)
    p2 = np.asarray(inputs["patch2"], np.float32).reshape(B, C, HW)
    bf = ml_dtypes.bfloat16
    Y = np.empty((B, CP, FW), bf)
    Y[:, :, 0:49] = p1[:, 0::2, :]
    Y[:, :, 49:98] = p2[:, 0::2, :]
    Y[:, :, 98:147] = p1[:, 1::2, :]
    Y[:, :, 147:196] = p2[:, 1::2, :]
    # pack 4 consecutive samples along each partition row -> 1568B runs
    Y4 = np.ascontiguousarray(
        Y.reshape(B // 4, 4, CP, FW).transpose(0, 2, 1, 3).reshape(
            B // 4, CP, 4 * FW
        )
    )
    wst2 = _build_wst2(np.asarray(inputs["w_bbox"], np.float32)).astype(bf)
    seld = np.zeros((128, 4), bf)
    for c in range(NCHAIN):
        for n in range(4):
            seld[32 * c + n, n] = 1
    seld[127, :] = np.asarray(inputs["b_bbox"], np.float32).astype(bf)
    in_maps = []
    for c in range(N_CORES):
        sl = slice(c * (BS // 4), (c + 1) * (BS // 4))
        in_maps.append(
            {
                "pp": np.ascontiguousarray(Y4[sl]),
                "wst2": wst2,
                "seld": seld,
                "onesd": np.ones((1, BS), bf),
            }
        )
    return in_maps


def _run(inputs, trace: bool = False):
    nc = build_nc()
    in_maps = _prep_inputs(inputs)
    res = run_bass_kernel_spmd(
        nc, in_maps, core_ids=list(range(N_CORES)), trace=trace
    )
    out = np.concatenate(
        [res.results[c]["out"].T for c in range(N_CORES)], axis=0
    ).astype(np.float32)
    return out, res


def kernel(**inputs) -> np.ndarray:
    out, _ = _run(inputs, trace=False)
    return out
